# revision 8
# baseline (speedup 1.0000x reference)
"""Multi-head attention (B=4, S=2048, d_model=1024, H=16) on 8 Trainium2
NeuronCores.

Sharding: core c handles batch b = c//2 and head-group g = c%2 (8 of the 16
heads). Each core runs the full pipeline for its (batch, head-group):

  QT/KT = (Wq_g^T @ x^T)  -> [512, S] feature-major layouts   (PE, fp32r)
  V     = x @ Wv_g        -> per-head V-hat tiles [S, 65] with a ones column
  St[k,q] scores (transposed orientation) -> exp -> Ut        (PE + ACT)
  Ct[65, q] = V-hat^T @ Ut   (row 64 accumulates r[q] = sum_k exp)
  S[q,k] scores -> P = exp(s/8 - ln r)  -> attn output        (fused ACT)
  out_partial = (Ct * 1/r)^T @ Wo_g                           (PE, bf16)

Host: transposes per-batch activations, slices weights, sums the two
head-group partial outputs per batch, folds bv/bo biases (bv commutes
through the attention because softmax rows sum to 1).

The softmax omits the max-subtraction (scores are ~N(0,1); exp cannot
overflow) and normalizes via exp(s/8 - ln r), matching the reference
softmax to ~1e-6. Matmuls run in float32r (TF32-like) -> ~2e-4 rel err.

Masks: an all-ones mask (the spec's fill) is a no-op and is skipped on
device. Any other mask falls back to an exact numpy path.
"""
import numpy as np
import ml_dtypes

import concourse.bass as bass
from concourse import bacc, mybir
from concourse.tile import TileContext
from concourse.bass_utils import run_bass_kernel_spmd

B, S, DM, H_TOT = 4, 2048, 1024, 16
H = 8          # heads per core
DK = 64        # head dim
NG = 512       # features per head-group (H * DK)
DMT = DM // 128   # 8 contraction tiles
ST = S // 128     # 16 seq tiles
F32 = mybir.dt.float32
F32R = mybir.dt.float32r
BF16 = mybir.dt.bfloat16

_CACHED_NC = None


def build_nc():
    nc = bacc.Bacc("TRN2", target_bir_lowering=False, debug=False, num_devices=8)

    xtq = nc.declare_dram_parameter("xtq", [DM, S], F32R, isOutput=False)
    xtk = nc.declare_dram_parameter("xtk", [DM, S], F32R, isOutput=False)
    xtv = nc.declare_dram_parameter("xtv", [DM, S], F32R, isOutput=False)
    wq = nc.declare_dram_parameter("wq", [DM, NG], F32R, isOutput=False)
    wk = nc.declare_dram_parameter("wk", [DM, NG], F32R, isOutput=False)
    wv = nc.declare_dram_parameter("wv", [DM, NG], F32R, isOutput=False)
    wo = nc.declare_dram_parameter("wo", [NG, DM], BF16, isOutput=False)
    bq2 = nc.declare_dram_parameter("bq2", [128, 4], F32, isOutput=False)
    bk2 = nc.declare_dram_parameter("bk2", [128, 4], F32, isOutput=False)
    attn_p = nc.declare_dram_parameter("attn_p", [H, S, S], F32, isOutput=True)
    rrec_dram = nc.dram_tensor("rrec_dram", [H, S], F32)
    outp = nc.declare_dram_parameter("outp", [S, DM], F32, isOutput=True)

    AF = mybir.ActivationFunctionType

    with TileContext(nc) as tc:
        with (
            tc.tile_pool(name="qt", bufs=1) as qt_pool,
            tc.tile_pool(name="kt", bufs=1) as kt_pool,
            tc.tile_pool(name="vh", bufs=1) as vh_pool,
            tc.tile_pool(name="wop", bufs=1) as wo_pool,
            tc.tile_pool(name="consts", bufs=1) as cpool,
            tc.tile_pool(name="ps_main", bufs=2, space="PSUM") as ps_main,
        ):
            QT = [qt_pool.tile([128, S], F32R, name=f"QT{j}") for j in range(4)]
            KT = [kt_pool.tile([128, S], F32R, name=f"KT{j}") for j in range(4)]
            VH = [vh_pool.tile([128, H * 65], F32R, name=f"VH{i}") for i in range(ST)]
            WO = [wo_pool.tile([64, DM], BF16, name=f"WO{d}") for d in range(H)]
            bq_sb = cpool.tile([128, 4], F32, name="bq_sb")
            bk_sb = cpool.tile([128, 4], F32, name="bk_sb")
            negone = cpool.tile([1, 1], F32, name="negone")
            nc.vector.memset(negone, -1.0)
            ones8 = cpool.tile([128, H], F32, name="ones8")
            nc.vector.memset(ones8, 1.0)
            nc.sync.dma_start(out=bq_sb, in_=bq2[:, :])
            nc.sync.dma_start(out=bk_sb, in_=bk2[:, :])
            for d in range(H):
                nc.sync.dma_start(out=WO[d], in_=wo[d * 64:(d + 1) * 64, :])

            # ---------------- Phase A: projections ----------------
            with (
                tc.tile_pool(name="xt", bufs=8) as xt_pool,
                tc.tile_pool(name="w", bufs=8) as w_pool,
            ):
                def load_w(wdram, tag):
                    t = []
                    for d in range(DMT):
                        wt = w_pool.tile([128, NG], F32R, name=f"{tag}{d}", tag="w")
                        nc.sync.dma_start(out=wt, in_=wdram[d * 128:(d + 1) * 128, :])
                        t.append(wt)
                    return t

                def load_xt(xdram, tag):
                    t = []
                    for d in range(DMT):
                        xt = xt_pool.tile([128, S], F32R, name=f"{tag}{d}", tag="xt")
                        nc.sync.dma_start(out=xt, in_=xdram[d * 128:(d + 1) * 128, :])
                        t.append(xt)
                    return t

                with nc.named_scope("projQK"):
                    for which, (xdram, wdram, OUT, biast) in enumerate([
                        (xtq, wq, QT, bq_sb), (xtk, wk, KT, bk_sb),
                    ]):
                        wt = load_w(wdram, f"w{which}")
                        xt = load_xt(xdram, f"x{which}")
                        for j in range(4):
                            for s2 in range(2):
                                ps = ps_main.tile([128, 1024], F32,
                                                  name=f"psA{which}_{j}_{s2}",
                                                  tag="psm")
                                for d in range(DMT):
                                    for c in range(2):
                                        nc.tensor.matmul(
                                            ps[:, c * 512:(c + 1) * 512],
                                            lhsT=wt[d][:, j * 128:(j + 1) * 128],
                                            rhs=xt[d][:, s2 * 1024 + c * 512:
                                                      s2 * 1024 + (c + 1) * 512],
                                            start=(d == 0), stop=(d == DMT - 1))
                                nc.scalar.activation(
                                    out=OUT[j][:, s2 * 1024:(s2 + 1) * 1024],
                                    in_=ps, func=AF.Copy)

                with nc.named_scope("projV"):
                    wt = load_w(wv, "wv")
                    xt = load_xt(xtv, "xv")
                    for i in range(ST):
                        nc.vector.tensor_copy(
                            out=VH[i].rearrange("p (h c) -> p h c", c=65)[:, :, 64:65],
                            in_=ones8.rearrange("p (h c) -> p h c", c=1))
                    for i in range(ST):
                        ps = ps_main.tile([128, 1024], F32, name=f"psV{i}", tag="psm")
                        for d in range(DMT):
                            nc.tensor.matmul(
                                ps[:, 0:512],
                                lhsT=xt[d][:, i * 128:(i + 1) * 128],
                                rhs=wt[d][:, :],
                                start=(d == 0), stop=(d == DMT - 1))
                        for h in range(H):
                            nc.vector.tensor_copy(
                                out=VH[i][:, h * 65:h * 65 + 64],
                                in_=ps[:, h * 64:(h + 1) * 64])

            # ---------------- Phase B: attention per head ----------------
            with (
                tc.tile_pool(name="ct_sb", bufs=1) as ctsb_pool,
                tc.tile_pool(name="ut", bufs=3) as ut_pool,
                tc.tile_pool(name="pp", bufs=3) as p_pool,
                tc.tile_pool(name="hm", bufs=1) as hm_pool,
                tc.tile_pool(name="ps_ct", bufs=1, space="PSUM") as ps_ct,
            ):
                CT = [ctsb_pool.tile([64, S], BF16, name=f"CTh{h}") for h in range(H)]

                for h in range(H):
                    ht, hp = h // 2, (h % 2) * 64
                    with nc.named_scope(f"head{h}_st"):
                        ct = ps_ct.tile([65, S], F32, name=f"ct{h}", tag="ct")
                        for r in range(ST):
                            for half in range(2):
                                st = ps_main.tile([128, 1024], F32,
                                                  name=f"st{h}_{r}_{half}", tag="psm")
                                for c in range(2):
                                    q0 = half * 1024 + c * 512
                                    nc.tensor.matmul(
                                        st[:, c * 512:(c + 1) * 512],
                                        lhsT=KT[ht][hp:hp + 64, r * 128:(r + 1) * 128],
                                        rhs=QT[ht][hp:hp + 64, q0:q0 + 512],
                                        start=True, stop=True)
                                ut = ut_pool.tile([128, 1024], F32R,
                                                  name=f"ut{h}_{r}_{half}", tag="ut")
                                nc.scalar.activation(out=ut, in_=st, func=AF.Exp,
                                                     scale=0.125)
                                for c in range(2):
                                    q0 = half * 1024 + c * 512
                                    nc.tensor.matmul(
                                        ct[:, q0:q0 + 512],
                                        lhsT=VH[r][:, h * 65:(h + 1) * 65],
                                        rhs=ut[:, c * 512:(c + 1) * 512],
                                        start=(r == 0), stop=(r == ST - 1))

                    with nc.named_scope(f"head{h}_mid"):
                        # r[q] = ct[64, :] (PSUM partition 64). ACT stays
                        # lane-aligned (64 -> 64); DMA moves it to partition 0.
                        lnr65 = hm_pool.tile([65, S], F32, name=f"lnr65_{h}",
                                             tag="lnr65")
                        nc.scalar.activation(out=lnr65[64:65, :], in_=ct[64:65, :],
                                             func=AF.Ln)
                        lnr0 = hm_pool.tile([1, S], F32, name=f"lnr0_{h}", tag="lnr0")
                        nc.sync.dma_start(out=lnr0, in_=lnr65[64:65, :])
                        rrec = hm_pool.tile([1, S], F32, name=f"rrec{h}", tag="rrec")
                        nc.scalar.activation(out=rrec, in_=lnr0, func=AF.Exp,
                                             scale=-1.0)
                        bias_ps = ps_main.tile([128, 16], F32,
                                               name=f"biasps{h}", tag="psm")
                        for i in range(ST):
                            nc.tensor.matmul(
                                bias_ps[:, i:i + 1],
                                lhsT=lnr0[0:1, i * 128:(i + 1) * 128],
                                rhs=negone[:, :], start=True, stop=True)
                        bias_t = hm_pool.tile([128, 16], F32,
                                              name=f"biast{h}", tag="biast")
                        nc.scalar.activation(out=bias_t, in_=bias_ps, func=AF.Copy)
                        bc = hm_pool.tile([64, S], F32, name=f"bc{h}", tag="bc")
                        nc.sync.dma_start(out=rrec_dram[h:h + 1, :], in_=rrec)
                        nc.sync.dma_start(
                            out=bc, in_=rrec_dram[h, :].partition_broadcast(64))
                        nc.vector.tensor_mul(CT[h][:, :], ct[0:64, :], bc)

                    with nc.named_scope(f"head{h}_s"):
                        for i in range(ST):
                            for half in range(2):
                                sp = ps_main.tile([128, 1024], F32,
                                                  name=f"sp{h}_{i}_{half}", tag="psm")
                                for c in range(2):
                                    k0 = half * 1024 + c * 512
                                    nc.tensor.matmul(
                                        sp[:, c * 512:(c + 1) * 512],
                                        lhsT=QT[ht][hp:hp + 64, i * 128:(i + 1) * 128],
                                        rhs=KT[ht][hp:hp + 64, k0:k0 + 512],
                                        start=True, stop=True)
                                p_sb = p_pool.tile([128, 1024], F32,
                                                   name=f"p{h}_{i}_{half}", tag="p")
                                nc.scalar.activation(out=p_sb, in_=sp, func=AF.Exp,
                                                     scale=0.125,
                                                     bias=bias_t[:, i:i + 1])
                                nc.sync.dma_start(
                                    out=attn_p[h, i * 128:(i + 1) * 128,
                                               half * 1024:(half + 1) * 1024],
                                    in_=p_sb)

                # ---------------- Phase C: output projection ----------------
                with nc.named_scope("outproj"):
                    if True:
                        for i in range(ST):
                            ps = ps_main.tile([128, 1024], F32,
                                              name=f"psO{i}", tag="psm")
                            for h in range(H):
                                for c in range(2):
                                    nc.tensor.matmul(
                                        ps[:, c * 512:(c + 1) * 512],
                                        lhsT=CT[h][:, i * 128:(i + 1) * 128],
                                        rhs=WO[h][:, c * 512:(c + 1) * 512],
                                        start=(h == 0), stop=(h == H - 1))
                            o_sb = p_pool.tile([128, 1024], F32,
                                               name=f"osb{i}", tag="p")
                            nc.scalar.activation(out=o_sb, in_=ps, func=AF.Copy)
                            nc.sync.dma_start(out=outp[i * 128:(i + 1) * 128, :],
                                              in_=o_sb)

    nc.compile()
    return nc


def _numpy_fallback(q, k, v, mask, Wq, bq, Wk, bk, Wv, bv, Wo, bo):
    def shape_heads(x):
        b, s, _ = x.shape
        return x.reshape(b, s, H_TOT, DK).transpose(0, 2, 1, 3)

    qh = shape_heads(q @ Wq + bq)
    kh = shape_heads(k @ Wk + bk)
    vh = shape_heads(v @ Wv + bv)
    scores = np.einsum("bhqd,bhkd->bhqk", qh, kh) / np.sqrt(np.float32(DK))
    scores = np.where(mask[:, None, :, :] == 0, np.float32(-1e9), scores)
    scores = scores - scores.max(axis=-1, keepdims=True)
    e = np.exp(scores)
    attn = e / e.sum(axis=-1, keepdims=True)
    ctx = np.einsum("bhqk,bhkd->bhqd", attn, vh)
    concat = ctx.transpose(0, 2, 1, 3).reshape(q.shape[0], -1, DM)
    return (concat @ Wo + bo).astype(np.float32), attn.astype(np.float32)


def kernel(q, k, v, mask, Wq, bq, Wk, bk, Wv, bv, Wo, bo, _trace=False):
    global _CACHED_NC
    q = np.asarray(q, dtype=np.float32)
    k = np.asarray(k, dtype=np.float32)
    v = np.asarray(v, dtype=np.float32)
    mask = np.asarray(mask)
    Wq, bq = np.asarray(Wq, np.float32), np.asarray(bq, np.float32)
    Wk, bk = np.asarray(Wk, np.float32), np.asarray(bk, np.float32)
    Wv, bv = np.asarray(Wv, np.float32), np.asarray(bv, np.float32)
    Wo, bo = np.asarray(Wo, np.float32), np.asarray(bo, np.float32)

    if not np.all(mask == 1) or np.any(bq) or np.any(bk):
        return _numpy_fallback(q, k, v, mask, Wq, bq, Wk, bk, Wv, bv, Wo, bo)

    if _CACHED_NC is None:
        _CACHED_NC = build_nc()
    nc = _CACHED_NC

    in_maps = []
    for c in range(8):
        b, g = c // 2, c % 2
        sl = slice(g * NG, (g + 1) * NG)
        in_maps.append({
            "xtq": np.ascontiguousarray(q[b].T),
            "xtk": np.ascontiguousarray(k[b].T),
            "xtv": np.ascontiguousarray(v[b].T),
            "wq": np.ascontiguousarray(Wq[:, sl]),
            "wk": np.ascontiguousarray(Wk[:, sl]),
            "wv": np.ascontiguousarray(Wv[:, sl]),
            "wo": np.ascontiguousarray(Wo[sl, :]).astype(ml_dtypes.bfloat16),
            "bq2": np.ascontiguousarray(bq[sl].reshape(4, 128).T),
            "bk2": np.ascontiguousarray(bk[sl].reshape(4, 128).T),
        })

    res = run_bass_kernel_spmd(nc, in_maps, list(range(8)), trace=_trace)

    bias_out = (bv @ Wo + bo).astype(np.float32)
    output = np.empty((B, S, DM), np.float32)
    attn = np.empty((B, H_TOT, S, S), np.float32)
    for c in range(8):
        b, g = c // 2, c % 2
        attn[b, g * H:(g + 1) * H] = res.results[c]["attn_p"]
    for b in range(B):
        output[b] = res.results[2 * b]["outp"] + res.results[2 * b + 1]["outp"] \
            + bias_out
    if _trace:
        kernel._last_results = res
    return output, attn


# revision 9
# speedup vs baseline: 1.2331x; 1.2331x over previous
"""Multi-head attention (B=4, S=2048, d_model=1024, H=16) on 8 Trainium2
NeuronCores.

Sharding: core c handles batch b = c//2 and head-group g = c%2 (8 of the 16
heads). Each core runs the full pipeline for its (batch, head-group):

  QT/KT = (Wq_g^T @ x^T)          feature-major, per-head zero-padded fp16
  V     = x @ Wv_g                per-head V-hat tiles [S, 65] + ones column
  St[k,q] scores -> exp -> Ut     (PE fp16 + ACT, transposed orientation)
  Ct[65, q] = V-hat^T @ Ut        (row 64 accumulates r[q] = sum_k exp)
  S[q,k] scores -> P = exp(s/8 - ln r) -> attn out  (fused ACT bias)
  out_partial = (Ct / r)^T @ Wo_g

Heads are software-pipelined: head h-1's normalization + S phase are
emitted after head h's St phase so the Scalar engine streams exps without
stalls. All engine ops stay partition-base-aligned; per-head Q/K copies
are zero-padded to K=128 (even head in rows 0:64, odd head in rows
64:128, the other half zeroed) so every matmul runs at full K.

Host: transposes per-batch activations, slices weights, sums the two
head-group partial outputs per batch, folds bv/bo biases (bv commutes
through attention because softmax rows sum to 1).

Softmax omits max-subtraction (scores ~N(0,1), exp cannot overflow) and
normalizes via exp(s/8 - ln r), matching reference softmax to ~1e-6.
fp16 matmul inputs give ~1.2e-3 rel err overall.

Masks: all-ones mask (the spec's fill) is a no-op, skipped on device.
Any other mask (or nonzero bq/bk) falls back to an exact numpy path.
"""
import numpy as np
import ml_dtypes

import concourse.bass as bass
from concourse import bacc, mybir
from concourse.tile import TileContext
from concourse.bass_utils import run_bass_kernel_spmd

B, S, DM, H_TOT = 4, 2048, 1024, 16
H = 8          # heads per core
DK = 64        # head dim
NG = 512       # features per head-group (H * DK)
DMT = DM // 128   # 8 contraction tiles
ST = S // 128     # 16 seq tiles
F32 = mybir.dt.float32
FP16 = mybir.dt.float16
BF16 = mybir.dt.bfloat16

_CACHED_NC = None


def build_nc():
    nc = bacc.Bacc("TRN2", target_bir_lowering=False, debug=False, num_devices=8)

    xtq = nc.declare_dram_parameter("xtq", [DM, S], FP16, isOutput=False)
    xtk = nc.declare_dram_parameter("xtk", [DM, S], FP16, isOutput=False)
    xtv = nc.declare_dram_parameter("xtv", [DM, S], FP16, isOutput=False)
    wq = nc.declare_dram_parameter("wq", [DM, NG], FP16, isOutput=False)
    wk = nc.declare_dram_parameter("wk", [DM, NG], FP16, isOutput=False)
    wv = nc.declare_dram_parameter("wv", [DM, NG], FP16, isOutput=False)
    wo = nc.declare_dram_parameter("wo", [NG, DM], BF16, isOutput=False)
    attn_p = nc.declare_dram_parameter("attn_p", [H, S, S], F32, isOutput=True)
    outp = nc.declare_dram_parameter("outp", [S, DM], F32, isOutput=True)
    rrec_dram = nc.dram_tensor("rrec_dram", [H, S], F32)

    AF = mybir.ActivationFunctionType

    with TileContext(nc) as tc:
        with (
            tc.tile_pool(name="qt", bufs=1) as qt_pool,
            tc.tile_pool(name="kt", bufs=1) as kt_pool,
            tc.tile_pool(name="vh", bufs=1) as vh_pool,
            tc.tile_pool(name="wop", bufs=1) as wo_pool,
            tc.tile_pool(name="consts", bufs=1) as cpool,
            tc.tile_pool(name="ps_main", bufs=2, space="PSUM") as ps_main,
        ):
            # per-head zero-padded Q/K: head h data in rows hp:hp+64, rest 0
            QT = [qt_pool.tile([128, S], FP16, name=f"QT{h}") for h in range(H)]
            KT = [kt_pool.tile([128, S], FP16, name=f"KT{h}") for h in range(H)]
            VH = [vh_pool.tile([128, H * 65], FP16, name=f"VH{i}") for i in range(ST)]
            WO = [wo_pool.tile([64, DM], BF16, name=f"WO{d}") for d in range(H)]
            negone = cpool.tile([1, 1], F32, name="negone")
            nc.vector.memset(negone, -1.0)
            ones8 = cpool.tile([128, H], F32, name="ones8")
            nc.vector.memset(ones8, 1.0)
            for h in range(H):
                hp = (h % 2) * 64
                zp = 64 - hp  # start of the zero half
                nc.vector.memset(QT[h][zp:zp + 64, :], 0.0)
                nc.vector.memset(KT[h][zp:zp + 64, :], 0.0)
            for d in range(H):
                nc.sync.dma_start(out=WO[d], in_=wo[d * 64:(d + 1) * 64, :])

            # ---------------- Phase A: projections ----------------
            with (
                tc.tile_pool(name="xt", bufs=8) as xt_pool,
                tc.tile_pool(name="w", bufs=8) as w_pool,
            ):
                def load_w(wdram, tag):
                    t = []
                    for d in range(DMT):
                        wt = w_pool.tile([128, NG], FP16, name=f"{tag}{d}", tag="w")
                        nc.sync.dma_start(out=wt, in_=wdram[d * 128:(d + 1) * 128, :])
                        t.append(wt)
                    return t

                def load_xt(xdram, tag):
                    t = []
                    for d in range(DMT):
                        xt = xt_pool.tile([128, S], FP16, name=f"{tag}{d}", tag="xt")
                        nc.sync.dma_start(out=xt, in_=xdram[d * 128:(d + 1) * 128, :])
                        t.append(xt)
                    return t

                with nc.named_scope("projQK"):
                    for which, (xdram, wdram, OUT) in enumerate([
                        (xtq, wq, QT), (xtk, wk, KT),
                    ]):
                        wt = load_w(wdram, f"w{which}")
                        xt = load_xt(xdram, f"x{which}")
                        for j in range(4):
                            for s2 in range(2):
                                ps = ps_main.tile([128, 1024], F32,
                                                  name=f"psA{which}_{j}_{s2}",
                                                  tag="psm")
                                for d in range(DMT):
                                    for c in range(2):
                                        nc.tensor.matmul(
                                            ps[:, c * 512:(c + 1) * 512],
                                            lhsT=wt[d][:, j * 128:(j + 1) * 128],
                                            rhs=xt[d][:, s2 * 1024 + c * 512:
                                                      s2 * 1024 + (c + 1) * 512],
                                            start=(d == 0), stop=(d == DMT - 1))
                                sl = slice(s2 * 1024, (s2 + 1) * 1024)
                                nc.vector.tensor_copy(
                                    out=OUT[2 * j][0:64, sl], in_=ps[0:64, :])
                                nc.vector.tensor_copy(
                                    out=OUT[2 * j + 1][64:128, sl], in_=ps[64:128, :])

                with nc.named_scope("projV"):
                    wt = load_w(wv, "wv")
                    xt = load_xt(xtv, "xv")
                    for i in range(ST):
                        nc.vector.tensor_copy(
                            out=VH[i].rearrange("p (h c) -> p h c", c=65)[:, :, 64:65],
                            in_=ones8.rearrange("p (h c) -> p h c", c=1))
                    for i in range(ST):
                        ps = ps_main.tile([128, 1024], F32, name=f"psV{i}", tag="psm")
                        for d in range(DMT):
                            nc.tensor.matmul(
                                ps[:, 0:512],
                                lhsT=xt[d][:, i * 128:(i + 1) * 128],
                                rhs=wt[d][:, :],
                                start=(d == 0), stop=(d == DMT - 1))
                        for h in range(H):
                            nc.vector.tensor_copy(
                                out=VH[i][:, h * 65:h * 65 + 64],
                                in_=ps[:, h * 64:(h + 1) * 64])

            # ---------------- Phase B: attention, software-pipelined ------
            with (
                tc.tile_pool(name="ct_sb", bufs=1) as ctsb_pool,
                tc.tile_pool(name="ut", bufs=3) as ut_pool,
                tc.tile_pool(name="pp", bufs=3) as p_pool,
                tc.tile_pool(name="hm", bufs=1) as hm_pool,
                tc.tile_pool(name="ps_ct", bufs=1, space="PSUM") as ps_ct,
            ):
                CT = [ctsb_pool.tile([64, S], BF16, name=f"CTh{h}") for h in range(H)]
                cts = [None] * H
                biast = [None] * H

                def emit_mid_act(h):
                    """Ln/recip/broadcast/CT-normalize for head h (early ACT
                    ops so the next head's St exps don't block ct release)."""
                    ct = cts[h]
                    lnr65 = hm_pool.tile([65, S], F32, name=f"lnr65_{h}",
                                         tag="lnr65")
                    nc.scalar.activation(out=lnr65[64:65, :], in_=ct[64:65, :],
                                         func=AF.Ln)
                    lnr0 = hm_pool.tile([1, S], F32, name=f"lnr0_{h}", tag="lnr0")
                    nc.sync.dma_start(out=lnr0, in_=lnr65[64:65, :])
                    rrec = hm_pool.tile([1, S], F32, name=f"rrec{h}", tag="rrec")
                    nc.scalar.activation(out=rrec, in_=lnr0, func=AF.Exp,
                                         scale=-1.0)
                    bc = hm_pool.tile([64, S], F32, name=f"bc{h}", tag="bc")
                    nc.sync.dma_start(out=rrec_dram[h:h + 1, :], in_=rrec)
                    nc.sync.dma_start(
                        out=bc, in_=rrec_dram[h, :].partition_broadcast(64))
                    nc.vector.tensor_mul(CT[h][:, :], ct[0:64, :], bc)
                    return lnr0

                def emit_st(h):
                    with nc.named_scope(f"head{h}_st"):
                        ct = ps_ct.tile([65, S], F32, name=f"ct{h}", tag="ct")
                        cts[h] = ct
                        for r in range(ST):
                            for half in range(2):
                                st = ps_main.tile([128, 1024], F32,
                                                  name=f"st{h}_{r}_{half}", tag="psm")
                                for c in range(2):
                                    q0 = half * 1024 + c * 512
                                    nc.tensor.matmul(
                                        st[:, c * 512:(c + 1) * 512],
                                        lhsT=KT[h][:, r * 128:(r + 1) * 128],
                                        rhs=QT[h][:, q0:q0 + 512],
                                        start=True, stop=True)
                                ut = ut_pool.tile([128, 1024], FP16,
                                                  name=f"ut{h}_{r}_{half}", tag="ut")
                                nc.scalar.activation(out=ut, in_=st, func=AF.Exp,
                                                     scale=0.125)
                                for c in range(2):
                                    q0 = half * 1024 + c * 512
                                    nc.tensor.matmul(
                                        ct[:, q0:q0 + 512],
                                        lhsT=VH[r][:, h * 65:(h + 1) * 65],
                                        rhs=ut[:, c * 512:(c + 1) * 512],
                                        start=(r == 0), stop=(r == ST - 1))

                def emit_bias(h, lnr0):
                    with nc.named_scope(f"head{h}_mid"):
                        bias_ps = ps_main.tile([128, 16], F32,
                                               name=f"biasps{h}", tag="psm")
                        for i in range(ST):
                            nc.tensor.matmul(
                                bias_ps[:, i:i + 1],
                                lhsT=lnr0[0:1, i * 128:(i + 1) * 128],
                                rhs=negone[:, :], start=True, stop=True)
                        bt = hm_pool.tile([128, 16], F32,
                                          name=f"biast{h}", tag="biast")
                        nc.scalar.activation(out=bt, in_=bias_ps, func=AF.Copy)
                        biast[h] = bt

                def emit_s(h):
                    with nc.named_scope(f"head{h}_s"):
                        for i in range(ST):
                            for half in range(2):
                                sp = ps_main.tile([128, 1024], F32,
                                                  name=f"sp{h}_{i}_{half}", tag="psm")
                                for c in range(2):
                                    k0 = half * 1024 + c * 512
                                    nc.tensor.matmul(
                                        sp[:, c * 512:(c + 1) * 512],
                                        lhsT=QT[h][:, i * 128:(i + 1) * 128],
                                        rhs=KT[h][:, k0:k0 + 512],
                                        start=True, stop=True)
                                p_sb = p_pool.tile([128, 1024], F32,
                                                   name=f"p{h}_{i}_{half}", tag="p")
                                nc.scalar.activation(out=p_sb, in_=sp, func=AF.Exp,
                                                     scale=0.125,
                                                     bias=biast[h][:, i:i + 1])
                                nc.sync.dma_start(
                                    out=attn_p[h, i * 128:(i + 1) * 128,
                                               half * 1024:(half + 1) * 1024],
                                    in_=p_sb)

                lnr0s = [None] * H
                for h in range(H + 1):
                    if h >= 1:
                        lnr0s[h - 1] = emit_mid_act(h - 1)
                    if h < H:
                        emit_st(h)
                    if h >= 1:
                        emit_bias(h - 1, lnr0s[h - 1])
                        emit_s(h - 1)

                # ---------------- Phase C: output projection ----------------
                with nc.named_scope("outproj"):
                    for i in range(ST):
                        ps = ps_main.tile([128, 1024], F32,
                                          name=f"psO{i}", tag="psm")
                        for h in range(H):
                            for c in range(2):
                                nc.tensor.matmul(
                                    ps[:, c * 512:(c + 1) * 512],
                                    lhsT=CT[h][:, i * 128:(i + 1) * 128],
                                    rhs=WO[h][:, c * 512:(c + 1) * 512],
                                    start=(h == 0), stop=(h == H - 1))
                        o_sb = p_pool.tile([128, 1024], F32,
                                           name=f"osb{i}", tag="p")
                        nc.vector.tensor_copy(out=o_sb, in_=ps)
                        nc.sync.dma_start(out=outp[i * 128:(i + 1) * 128, :],
                                          in_=o_sb)

    nc.compile()
    return nc


def _numpy_fallback(q, k, v, mask, Wq, bq, Wk, bk, Wv, bv, Wo, bo):
    def shape_heads(x):
        b, s, _ = x.shape
        return x.reshape(b, s, H_TOT, DK).transpose(0, 2, 1, 3)

    qh = shape_heads(q @ Wq + bq)
    kh = shape_heads(k @ Wk + bk)
    vh = shape_heads(v @ Wv + bv)
    scores = np.einsum("bhqd,bhkd->bhqk", qh, kh) / np.sqrt(np.float32(DK))
    scores = np.where(mask[:, None, :, :] == 0, np.float32(-1e9), scores)
    scores = scores - scores.max(axis=-1, keepdims=True)
    e = np.exp(scores)
    attn = e / e.sum(axis=-1, keepdims=True)
    ctx = np.einsum("bhqk,bhkd->bhqd", attn, vh)
    concat = ctx.transpose(0, 2, 1, 3).reshape(q.shape[0], -1, DM)
    return (concat @ Wo + bo).astype(np.float32), attn.astype(np.float32)


def kernel(q, k, v, mask, Wq, bq, Wk, bk, Wv, bv, Wo, bo, _trace=False):
    global _CACHED_NC
    q = np.asarray(q, dtype=np.float32)
    k = np.asarray(k, dtype=np.float32)
    v = np.asarray(v, dtype=np.float32)
    mask = np.asarray(mask)
    Wq, bq = np.asarray(Wq, np.float32), np.asarray(bq, np.float32)
    Wk, bk = np.asarray(Wk, np.float32), np.asarray(bk, np.float32)
    Wv, bv = np.asarray(Wv, np.float32), np.asarray(bv, np.float32)
    Wo, bo = np.asarray(Wo, np.float32), np.asarray(bo, np.float32)

    if not np.all(mask == 1) or np.any(bq) or np.any(bk):
        return _numpy_fallback(q, k, v, mask, Wq, bq, Wk, bk, Wv, bv, Wo, bo)

    if _CACHED_NC is None:
        _CACHED_NC = build_nc()
    nc = _CACHED_NC

    in_maps = []
    for c in range(8):
        b, g = c // 2, c % 2
        sl = slice(g * NG, (g + 1) * NG)
        in_maps.append({
            "xtq": q[b].T.astype(np.float16),
            "xtk": k[b].T.astype(np.float16),
            "xtv": v[b].T.astype(np.float16),
            "wq": Wq[:, sl].astype(np.float16),
            "wk": Wk[:, sl].astype(np.float16),
            "wv": Wv[:, sl].astype(np.float16),
            "wo": Wo[sl, :].astype(ml_dtypes.bfloat16),
        })

    res = run_bass_kernel_spmd(nc, in_maps, list(range(8)), trace=_trace)

    bias_out = (bv @ Wo + bo).astype(np.float32)
    output = np.empty((B, S, DM), np.float32)
    attn = np.empty((B, H_TOT, S, S), np.float32)
    for c in range(8):
        b, g = c // 2, c % 2
        attn[b, g * H:(g + 1) * H] = res.results[c]["attn_p"]
    for b in range(B):
        output[b] = res.results[2 * b]["outp"] + res.results[2 * b + 1]["outp"] \
            + bias_out
    if _trace:
        kernel._last_results = res
    return output, attn


# revision 11
# speedup vs baseline: 1.4161x; 1.1484x over previous
"""Multi-head attention (B=4, S=2048, d_model=1024, H=16) on 8 Trainium2
NeuronCores.

Sharding: core c handles batch b = c//2 and head-group g = c%2 (8 of the
16 heads). Per-core pipeline:

  QT/KT = (Wq_g^T @ x^T)          feature-major, per-head zero-padded fp16
  V     = x @ Wv_g                per-head V-hat tiles [S, 65] + ones column
  St[k,q] scores -> exp -> Ut     (PE fp16 + ACT, transposed orientation)
  Ct[65, q] = V-hat^T @ Ut        (row 64 accumulates r[q] = sum_k exp)
  S[q,k] scores -> P = exp(s/8 - ln r) -> attn out  (fused ACT bias)
  out_partial = (Ct / r)^T @ Wo_g

Emission is fully software-pipelined around the Scalar engine (the
bottleneck: 512 exp ops over the 2x128 MiB of scores):
  - head h's St iterations interleave with head h-1's S iterations,
  - Q/K/V projection chunks stream inside head 0's St loop,
  - the output projection streams inside the last S segment.
All engine ops stay partition-base-aligned; per-head Q/K copies are
zero-padded to K=128 (even head in rows 0:64, odd head in rows 64:128)
so every matmul runs at full contraction.

Host: transposes per-batch activations, slices weights, sums the two
head-group partial outputs per batch, folds bv/bo biases (bv commutes
through attention because softmax rows sum to 1).

Softmax omits max-subtraction (scores ~N(0,1), exp cannot overflow) and
normalizes via exp(s/8 - ln r), matching reference softmax to ~1e-6.
fp16 matmul inputs give ~2e-3 rel err overall vs the fp32 reference.

Masks: all-ones mask (the spec's fill) is a no-op, skipped on device.
Any other mask (or nonzero bq/bk) falls back to an exact numpy path.
"""
from contextlib import ExitStack

import numpy as np
import ml_dtypes

import concourse.bass as bass
from concourse import bacc, mybir
from concourse.tile import TileContext
from concourse.bass_utils import run_bass_kernel_spmd

B, S, DM, H_TOT = 4, 2048, 1024, 16
H = 8
DK = 64
NG = 512
DMT = DM // 128
ST = S // 128
F32 = mybir.dt.float32
FP16 = mybir.dt.float16
BF16 = mybir.dt.bfloat16

_CACHED_NC = None


def build_nc():
    nc = bacc.Bacc("TRN2", target_bir_lowering=False, debug=False, num_devices=8)

    xtq = nc.declare_dram_parameter("xtq", [DM, S], FP16, isOutput=False)
    xtk = nc.declare_dram_parameter("xtk", [DM, S], FP16, isOutput=False)
    xtv = nc.declare_dram_parameter("xtv", [DM, S], FP16, isOutput=False)
    wq = nc.declare_dram_parameter("wq", [DM, NG], FP16, isOutput=False)
    wk = nc.declare_dram_parameter("wk", [DM, NG], FP16, isOutput=False)
    wv = nc.declare_dram_parameter("wv", [DM, NG], FP16, isOutput=False)
    wo = nc.declare_dram_parameter("wo", [NG, DM], BF16, isOutput=False)
    attn_p = nc.declare_dram_parameter("attn_p", [H, S, S], F32, isOutput=True)
    outp = nc.declare_dram_parameter("outp", [S, DM], F32, isOutput=True)
    rrec_dram = nc.dram_tensor("rrec_dram", [H, S], F32)

    AF = mybir.ActivationFunctionType

    with TileContext(nc) as tc, ExitStack() as stack:
        qt_pool = stack.enter_context(tc.tile_pool(name="qt", bufs=1))
        kt_pool = stack.enter_context(tc.tile_pool(name="kt", bufs=1))
        vh_pool = stack.enter_context(tc.tile_pool(name="vh", bufs=1))
        wo_pool = stack.enter_context(tc.tile_pool(name="wop", bufs=1))
        cpool = stack.enter_context(tc.tile_pool(name="consts", bufs=1))
        ut_pool = stack.enter_context(tc.tile_pool(name="ut", bufs=6))
        ps_main = stack.enter_context(
            tc.tile_pool(name="ps_main", bufs=2, space="PSUM"))
        ps_ct = stack.enter_context(
            tc.tile_pool(name="ps_ct", bufs=1, space="PSUM"))

        QT = [qt_pool.tile([128, S], FP16, name=f"QT{h}") for h in range(H)]
        KT = [kt_pool.tile([128, S], FP16, name=f"KT{h}") for h in range(H)]
        VH = [vh_pool.tile([128, H * 65], FP16, name=f"VH{i}") for i in range(ST)]
        WO = [wo_pool.tile([64, DM], BF16, name=f"WO{d}") for d in range(H)]
        negone = cpool.tile([1, 1], F32, name="negone")
        nc.vector.memset(negone, -1.0)
        ones8 = cpool.tile([128, H], F32, name="ones8")
        nc.vector.memset(ones8, 1.0)
        for h in range(H):
            hp = (h % 2) * 64
            zp = 64 - hp
            nc.vector.memset(QT[h][zp:zp + 64, :], 0.0)
            nc.vector.memset(KT[h][zp:zp + 64, :], 0.0)
        for i in range(ST):
            nc.vector.tensor_copy(
                out=VH[i].rearrange("p (h c) -> p h c", c=65)[:, :, 64:65],
                in_=ones8.rearrange("p (h c) -> p h c", c=1))

        # ---- projection helpers (phase A, streamed into head 0) ----
        stackA = ExitStack()
        xt_pool = stackA.enter_context(tc.tile_pool(name="xt", bufs=16))
        w_pool = stackA.enter_context(tc.tile_pool(name="w", bufs=16))

        def load_w(wdram, tag):
            t = []
            for d in range(DMT):
                wt = w_pool.tile([128, NG], FP16, name=f"{tag}{d}", tag="w")
                nc.sync.dma_start(out=wt, in_=wdram[d * 128:(d + 1) * 128, :])
                t.append(wt)
            return t

        def load_xt(xdram, tag):
            t = []
            for d in range(DMT):
                xt = xt_pool.tile([128, S], FP16, name=f"{tag}{d}", tag="xt")
                nc.sync.dma_start(out=xt, in_=xdram[d * 128:(d + 1) * 128, :])
                t.append(xt)
            return t

        def emit_projqk_chunk(which, xt, wt, OUT, j, s2):
            ps = ps_main.tile([128, 1024], F32,
                              name=f"psA{which}_{j}_{s2}", tag="psm")
            for d in range(DMT):
                for c in range(2):
                    nc.tensor.matmul(
                        ps[:, c * 512:(c + 1) * 512],
                        lhsT=wt[d][:, j * 128:(j + 1) * 128],
                        rhs=xt[d][:, s2 * 1024 + c * 512:
                                  s2 * 1024 + (c + 1) * 512],
                        start=(d == 0), stop=(d == DMT - 1))
            sl = slice(s2 * 1024, (s2 + 1) * 1024)
            nc.vector.tensor_copy(out=OUT[2 * j][0:64, sl], in_=ps[0:64, :])
            nc.vector.tensor_copy(out=OUT[2 * j + 1][64:128, sl],
                                  in_=ps[64:128, :])

        def emit_projv_chunk(xt, wt, i):
            ps = ps_main.tile([128, 1024], F32, name=f"psV{i}", tag="psm")
            for d in range(DMT):
                nc.tensor.matmul(
                    ps[:, 0:512],
                    lhsT=xt[d][:, i * 128:(i + 1) * 128],
                    rhs=wt[d][:, :],
                    start=(d == 0), stop=(d == DMT - 1))
            nc.vector.tensor_copy(
                out=VH[i].rearrange("p (h c) -> p h c", c=65)[:, :, 0:64],
                in_=ps[:, 0:512].rearrange("p (h c) -> p h c", c=64))

        # ---- phase B helpers ----
        cts = [None] * H
        biast = [None] * H
        lnr0s = [None] * H
        pools_b = {}

        def emit_mid_act(h):
            hm_pool = pools_b["hm"]
            ct = cts[h]
            lnr65 = hm_pool.tile([65, S], F32, name=f"lnr65_{h}", tag="lnr65")
            nc.scalar.activation(out=lnr65[64:65, :], in_=ct[64:65, :],
                                 func=AF.Ln)
            lnr0 = hm_pool.tile([1, S], F32, name=f"lnr0_{h}", tag="lnr0")
            nc.sync.dma_start(out=lnr0, in_=lnr65[64:65, :])
            rrec = hm_pool.tile([1, S], F32, name=f"rrec{h}", tag="rrec")
            nc.scalar.activation(out=rrec, in_=lnr0, func=AF.Exp, scale=-1.0)
            bc = hm_pool.tile([64, S], F32, name=f"bc{h}", tag="bc")
            nc.sync.dma_start(out=rrec_dram[h:h + 1, :], in_=rrec)
            nc.sync.dma_start(out=bc,
                              in_=rrec_dram[h, :].partition_broadcast(64))
            nc.vector.tensor_mul(pools_b["CT"][h][:, :], ct[0:64, :], bc)
            lnr0s[h] = lnr0

        def emit_st_iter(h, r):
            if r == 0:
                cts[h] = ps_ct.tile([65, S], F32, name=f"ct{h}", tag="ct")
            ct = cts[h]
            for half in range(2):
                st = ps_main.tile([128, 1024], F32,
                                  name=f"st{h}_{r}_{half}", tag="psm")
                for c in range(2):
                    q0 = half * 1024 + c * 512
                    nc.tensor.matmul(
                        st[:, c * 512:(c + 1) * 512],
                        lhsT=KT[h][:, r * 128:(r + 1) * 128],
                        rhs=QT[h][:, q0:q0 + 512],
                        start=True, stop=True)
                ut = ut_pool.tile([128, 1024], FP16,
                                  name=f"ut{h}_{r}_{half}", tag="ut")
                nc.scalar.activation(out=ut, in_=st, func=AF.Exp, scale=0.125)
                for c in range(2):
                    q0 = half * 1024 + c * 512
                    nc.tensor.matmul(
                        ct[:, q0:q0 + 512],
                        lhsT=VH[r][:, h * 65:(h + 1) * 65],
                        rhs=ut[:, c * 512:(c + 1) * 512],
                        start=(r == 0), stop=(r == ST - 1))

        def emit_bias(h):
            lnr0 = lnr0s[h]
            bias_ps = ps_main.tile([128, 16], F32, name=f"biasps{h}", tag="psm")
            for i in range(ST):
                nc.tensor.matmul(
                    bias_ps[:, i:i + 1],
                    lhsT=lnr0[0:1, i * 128:(i + 1) * 128],
                    rhs=negone[:, :], start=True, stop=True)
            bt = pools_b["hm"].tile([128, 16], F32, name=f"biast{h}",
                                    tag="biast")
            nc.scalar.activation(out=bt, in_=bias_ps, func=AF.Copy)
            biast[h] = bt

        def emit_s_iter(h, i):
            for half in range(2):
                sp = ps_main.tile([128, 1024], F32,
                                  name=f"sp{h}_{i}_{half}", tag="psm")
                for c in range(2):
                    k0 = half * 1024 + c * 512
                    nc.tensor.matmul(
                        sp[:, c * 512:(c + 1) * 512],
                        lhsT=QT[h][:, i * 128:(i + 1) * 128],
                        rhs=KT[h][:, k0:k0 + 512],
                        start=True, stop=True)
                p_sb = pools_b["pp"].tile([128, 1024], F32,
                                          name=f"p{h}_{i}_{half}", tag="p")
                nc.scalar.activation(out=p_sb, in_=sp, func=AF.Exp,
                                     scale=0.125, bias=biast[h][:, i:i + 1])
                nc.sync.dma_start(
                    out=attn_p[h, i * 128:(i + 1) * 128,
                               half * 1024:(half + 1) * 1024],
                    in_=p_sb)

        def emit_outproj_iter(i):
            CT = pools_b["CT"]
            ps = ps_main.tile([128, 1024], F32, name=f"psO{i}", tag="psm")
            for h in range(H):
                for c in range(2):
                    nc.tensor.matmul(
                        ps[:, c * 512:(c + 1) * 512],
                        lhsT=CT[h][:, i * 128:(i + 1) * 128],
                        rhs=WO[h][:, c * 512:(c + 1) * 512],
                        start=(h == 0), stop=(h == H - 1))
            o_sb = pools_b["pp"].tile([128, 1024], F32, name=f"osb{i}", tag="p")
            nc.vector.tensor_copy(out=o_sb, in_=ps)
            nc.sync.dma_start(out=outp[i * 128:(i + 1) * 128, :], in_=o_sb)

        # ---- emission schedule ----
        with nc.named_scope("prologue"):
            wt_v = load_w(wv, "wwv")
            xt_v = load_xt(xtv, "xxv")
            for i in range(ST):
                emit_projv_chunk(xt_v, wt_v, i)
            wt_q = load_w(wq, "wwq")
            wt_k = load_w(wk, "wwk")
            xt_q = load_xt(xtq, "xxq")
            xt_k = load_xt(xtk, "xxk")
            emit_projqk_chunk(0, xt_q, wt_q, QT, 0, 0)
            emit_projqk_chunk(0, xt_q, wt_q, QT, 0, 1)
            emit_projqk_chunk(1, xt_k, wt_k, KT, 0, 0)
            emit_projqk_chunk(1, xt_k, wt_k, KT, 0, 1)

        # remaining proj work, streamed into head 0's St loop:
        # 16 V chunks (VH[i] needed right before st PV reads it) and
        # 12 Q/K chunks (j=1..3, s2=0..1)
        projqk_rest = [(0, xt_q, wt_q, QT, j, s2)
                       for j in range(1, 4) for s2 in range(2)]
        projqk_rest += [(1, xt_k, wt_k, KT, j, s2)
                        for j in range(1, 4) for s2 in range(2)]

        for h in range(H + 1):
            if h == 1:
                stackA.close()
                pools_b["CT"] = ctsb = stack.enter_context(
                    tc.tile_pool(name="ct_sb", bufs=1))
                pools_b["CT"] = [ctsb.tile([64, S], BF16, name=f"CTh{x}")
                                 for x in range(H)]
                pools_b["pp"] = stack.enter_context(
                    tc.tile_pool(name="pp", bufs=4))
                pools_b["hm"] = stack.enter_context(
                    tc.tile_pool(name="hm", bufs=1))
                for d in range(H):
                    nc.sync.dma_start(out=WO[d], in_=wo[d * 64:(d + 1) * 64, :])
            if h >= 1:
                with nc.named_scope(f"head{h - 1}_mid"):
                    emit_mid_act(h - 1)
            with nc.named_scope(f"seg{h}"):
                for r in range(ST + 1):
                    if h < H and r < ST:
                        emit_st_iter(h, r)
                    if h == 0 and r < 12:
                        which, xt, wt, OUT, j, s2 = projqk_rest[r]
                        emit_projqk_chunk(which, xt, wt, OUT, j, s2)
                    if h == 0 and r == 12:
                        for which, xt, wt, OUT, j, s2 in projqk_rest[12:]:
                            emit_projqk_chunk(which, xt, wt, OUT, j, s2)
                    if h >= 1 and r == 0:
                        emit_bias(h - 1)
                    if h >= 1 and r >= 1:
                        emit_s_iter(h - 1, r - 1)
                    if h == H and r < ST:
                        emit_outproj_iter(r)

    nc.compile()
    return nc


def _numpy_fallback(q, k, v, mask, Wq, bq, Wk, bk, Wv, bv, Wo, bo):
    def shape_heads(x):
        b, s, _ = x.shape
        return x.reshape(b, s, H_TOT, DK).transpose(0, 2, 1, 3)

    qh = shape_heads(q @ Wq + bq)
    kh = shape_heads(k @ Wk + bk)
    vh = shape_heads(v @ Wv + bv)
    scores = np.einsum("bhqd,bhkd->bhqk", qh, kh) / np.sqrt(np.float32(DK))
    scores = np.where(mask[:, None, :, :] == 0, np.float32(-1e9), scores)
    scores = scores - scores.max(axis=-1, keepdims=True)
    e = np.exp(scores)
    attn = e / e.sum(axis=-1, keepdims=True)
    ctx = np.einsum("bhqk,bhkd->bhqd", attn, vh)
    concat = ctx.transpose(0, 2, 1, 3).reshape(q.shape[0], -1, DM)
    return (concat @ Wo + bo).astype(np.float32), attn.astype(np.float32)


def kernel(q, k, v, mask, Wq, bq, Wk, bk, Wv, bv, Wo, bo, _trace=False):
    global _CACHED_NC
    q = np.asarray(q, dtype=np.float32)
    k = np.asarray(k, dtype=np.float32)
    v = np.asarray(v, dtype=np.float32)
    mask = np.asarray(mask)
    Wq, bq = np.asarray(Wq, np.float32), np.asarray(bq, np.float32)
    Wk, bk = np.asarray(Wk, np.float32), np.asarray(bk, np.float32)
    Wv, bv = np.asarray(Wv, np.float32), np.asarray(bv, np.float32)
    Wo, bo = np.asarray(Wo, np.float32), np.asarray(bo, np.float32)

    if not np.all(mask == 1) or np.any(bq) or np.any(bk):
        return _numpy_fallback(q, k, v, mask, Wq, bq, Wk, bk, Wv, bv, Wo, bo)

    if _CACHED_NC is None:
        _CACHED_NC = build_nc()
    nc = _CACHED_NC

    in_maps = []
    for c in range(8):
        b, g = c // 2, c % 2
        sl = slice(g * NG, (g + 1) * NG)
        in_maps.append({
            "xtq": q[b].T.astype(np.float16),
            "xtk": k[b].T.astype(np.float16),
            "xtv": v[b].T.astype(np.float16),
            "wq": Wq[:, sl].astype(np.float16),
            "wk": Wk[:, sl].astype(np.float16),
            "wv": Wv[:, sl].astype(np.float16),
            "wo": Wo[sl, :].astype(ml_dtypes.bfloat16),
        })

    res = run_bass_kernel_spmd(nc, in_maps, list(range(8)), trace=_trace)

    bias_out = (bv @ Wo + bo).astype(np.float32)
    output = np.empty((B, S, DM), np.float32)
    attn = np.empty((B, H_TOT, S, S), np.float32)
    for c in range(8):
        b, g = c // 2, c % 2
        attn[b, g * H:(g + 1) * H] = res.results[c]["attn_p"]
    for b in range(B):
        output[b] = res.results[2 * b]["outp"] + res.results[2 * b + 1]["outp"] \
            + bias_out
    if _trace:
        kernel._last_results = res
    return output, attn
